# revision 1
# baseline (speedup 1.0000x reference)
"""Trainium2 Bass kernel for nn_NeuralODECNN (RK4 neural-ODE CNN forward).

Self-contained: hardcodes all shapes. Data-parallel over batch across 8
NeuronCores (16 images per core); all params replicated.

Per-core on-chip layouts (B_local=16 images, j = g*4 + s, g=group, s=slot):
  folded  [128 = g*32 + c, 4096 = s*1024 + y*32 + x]   (z / k / x tensors, c<32)
  wide    [128 = channel,  16384 = j*1024 + y*32 + x]  (h1/h2, 128 channels)
  h1pad   [128, 16, 34, 34]  zero-padded per image for the 3x3 conv taps

conv1 (32ch -> 128ch 1x1): row-tiled K=32 matmuls (row group = image group)
conv2 (3x3 SAME): 9 accumulating matmuls per output chunk, shifted AP taps
conv3 (128ch -> 32ch 1x1): col-tiled M=32 matmuls writing folded psum
softplus(x) = Ln(Exp(x)+1) on ScalarE (exp+ln live in one ACT table set)
tanh(x)     = 2*recip(1+Exp(-2x)) - 1 (Exp on ScalarE, recip on VectorE)
t-channel of conv1 folded into per-eval biases (b1e) on the host.
readout: DMA refold z -> [128 = t*32+c, 16*256], 256 accumulating fp32
matmuls -> logits psum [10,16]; log-softmax + onehot loss + argmax-accuracy
on device; host sums the 8 per-core (loss_sum, acc_sum) pairs.
"""

import os
from contextlib import ExitStack

import ml_dtypes
import numpy as np

import concourse.bacc as bacc
import concourse.mybir as mybir
import concourse.tile as tile
from concourse.bass_utils import run_bass_kernel_spmd

F32 = mybir.dt.float32
AF = mybir.ActivationFunctionType

NCORES = 8
BL = 16          # images per core
STEPS = 4        # RK4 steps (= pieces, STEPS_PER_PIECE=1, dt=1)
DT_NAME = os.environ.get("ODE_DT", "bf16")   # bf16 | fp32r | fp32
LAG = 2          # conv2 image lag behind conv1 within an eval

# eval schedule: step i evals use t = i + {0,.5,.5,1}, piece = i (k1..k3) or
# min(i+1,3) (k4, since floor(i+1) indexes the next piece)
_EVAL_TP = [(i + dt, i if k < 3 else min(i + 1, 3))
            for i in range(4) for k, dt in enumerate((0.0, 0.5, 0.5, 1.0))]


def _mm_dtype():
    return {"bf16": mybir.dt.bfloat16, "fp32r": F32, "fp32": F32}[DT_NAME]


def build_nc(debug=False):
    DT = _mm_dtype()
    if DT_NAME == "fp32r":
        cast = lambda ap: ap.bitcast(mybir.dt.float32r)  # noqa: E731
    else:
        cast = lambda ap: ap  # noqa: E731

    nc = bacc.Bacc("TRN2")

    xf_d = nc.dram_tensor("xf", [128, 4096], DT, kind="ExternalInput")
    # conv1/aug weights are zero-padded to full K=128 (rows outside the
    # group's 32-partition strip are zero) — row-tiled partial-K matmuls
    # return garbage on this HW path, full-K costs the same N cycles.
    w1_d = nc.dram_tensor("w1s", [4, 4, 128, 128], DT, kind="ExternalInput")
    w2_d = nc.dram_tensor("w2s", [4, 9, 128, 128], DT, kind="ExternalInput")
    w3_d = nc.dram_tensor("w3s", [4, 128, 32], DT, kind="ExternalInput")
    aw_d = nc.dram_tensor("augw", [4, 128, 32], DT, kind="ExternalInput")
    b1_d = nc.dram_tensor("b1e", [128, 16], F32, kind="ExternalInput")
    b2_d = nc.dram_tensor("b2s", [128, 4], F32, kind="ExternalInput")
    b3_d = nc.dram_tensor("b3s", [128, 4], F32, kind="ExternalInput")  # -2*b3 folded
    ab_d = nc.dram_tensor("augb", [128, 1], F32, kind="ExternalInput")
    ro_d = nc.dram_tensor("row", [128, 2560], F32, kind="ExternalInput")
    oh_d = nc.dram_tensor("oneh", [16, 10], F32, kind="ExternalInput")
    rb_d = nc.dram_tensor("rob", [16, 10], F32, kind="ExternalInput")
    out_d = nc.dram_tensor("outv", [2, 1], F32, kind="ExternalOutput")
    if debug:
        zf_d = nc.dram_tensor("zf", [128, 4096], F32, kind="ExternalOutput")
        lg_d = nc.dram_tensor("lg", [16, 10], F32, kind="ExternalOutput")

    with tile.TileContext(nc) as tc, ExitStack() as ctx:
        sing = ctx.enter_context(tc.tile_pool(name="sing", bufs=1))
        z = sing.tile([128, 4096], F32)
        zin = sing.tile([128, 4096], DT)
        acc = sing.tile([128, 4096], F32)
        w1b = sing.tile([128, 4, 4, 128], DT)
        w2b = sing.tile([128, 4, 9, 128], DT)
        w3b = sing.tile([128, 4, 32], DT)
        awb = sing.tile([128, 4, 32], DT)
        b1b = sing.tile([128, 16], F32)
        b2b = sing.tile([128, 4], F32)
        b3b = sing.tile([128, 4], F32)
        abb = sing.tile([128, 1], F32)

        nc.sync.dma_start(w1b[:], w1_d.rearrange("p g i m -> i p g m"))
        nc.sync.dma_start(w2b[:], w2_d.rearrange("p t i m -> i p t m"))
        nc.sync.dma_start(w3b[:], w3_d.rearrange("p i m -> i p m"))
        nc.sync.dma_start(awb[:], aw_d.rearrange("g i m -> i g m"))
        nc.sync.dma_start(b1b[:], b1_d[:])
        nc.sync.dma_start(b2b[:], b2_d[:])
        nc.sync.dma_start(b3b[:], b3_d[:])
        nc.sync.dma_start(abb[:], ab_d[:])

        with (
            tc.tile_pool(name="mid", bufs=1) as mid,
            tc.tile_pool(name="p1", bufs=1, space="PSUM") as p1p,
            tc.tile_pool(name="p2", bufs=2, space="PSUM") as p2p,
            tc.tile_pool(name="p3", bufs=2, space="PSUM") as p3p,
            tc.tile_pool(name="stg", bufs=3) as stg,
        ):
            h1pad = mid.tile([128, 16, 34, 34], DT)
            h2b = mid.tile([128, 16384], DT)
            e3b = mid.tile([128, 4096], F32)
            t0b = mid.tile([128, 4096], F32)
            xfb = mid.tile([128, 4096], DT)

            nc.sync.dma_start(xfb[:], xf_d[:])
            nc.vector.memset(h1pad[:], 0.0)

            # ---- augment: z0 = aug_W @ x + aug_b (col-tiled, zero-pad K) ----
            for s in range(4):
                ps = p1p.tile([128, 1024], F32, tag="ps1")
                for g in range(4):
                    for h in range(2):
                        n0 = s * 1024 + h * 512
                        nc.tensor.matmul(
                            ps[32 * g:32 * g + 32, h * 512:(h + 1) * 512],
                            cast(awb[:, g, :]),
                            cast(xfb[:, n0:n0 + 512]),
                            start=True, stop=True, tile_position=(0, 32 * g))
                sl = slice(s * 1024, (s + 1) * 1024)
                nc.scalar.activation(z[:, sl], ps[:], AF.Identity, bias=abb[:, 0:1])
                nc.vector.tensor_copy(zin[:, sl], z[:, sl])

            # ---- the 16 RK4 sub-evaluations ----
            def conv1(j, piece, eidx):
                g, s = j // 4, j % 4
                ps1 = p1p.tile([128, 1024], F32, tag="ps1")
                for h in range(2):
                    n0 = s * 1024 + h * 512
                    nc.tensor.matmul(
                        ps1[:, h * 512:(h + 1) * 512],
                        cast(w1b[:, piece, g, :]),
                        cast(zin[:, n0:n0 + 512]),
                        start=True, stop=True)
                st = stg.tile([128, 1024], F32, tag="st")
                nc.scalar.activation(st[:], ps1[:], AF.Exp,
                                     bias=b1b[:, eidx:eidx + 1])
                nc.scalar.activation(h1pad[:, j, 1:33, 1:33],
                                     st.rearrange("p (a b) -> p a b", b=32),
                                     AF.Ln, bias=1.0)

            def conv2(j, piece):
                ps2 = p2p.tile([128, 1024], F32, tag="ps2")
                for tap in range(9):
                    dy, dx = tap // 3, tap % 3
                    for h in range(2):
                        y0 = h * 16 + dy
                        nc.tensor.matmul(
                            ps2[:, h * 512:(h + 1) * 512],
                            cast(w2b[:, piece, tap, :]),
                            cast(h1pad[:, j, y0:y0 + 16, dx:dx + 32]),
                            start=(tap == 0), stop=(tap == 8))
                st = stg.tile([128, 1024], F32, tag="st2")
                nc.scalar.activation(st[:], ps2[:], AF.Exp,
                                     bias=b2b[:, piece:piece + 1])
                nc.scalar.activation(h2b[:, j * 1024:(j + 1) * 1024], st[:],
                                     AF.Ln, bias=1.0)

            def conv3(s, piece):
                for ch in (2 * s, 2 * s + 1):
                    ps3 = p3p.tile([128, 512], F32, tag="ps3")
                    for g in range(4):
                        n0 = (g * 4 + ch // 2) * 1024 + (ch % 2) * 512
                        nc.tensor.matmul(
                            ps3[32 * g:32 * g + 32, :],
                            cast(w3b[:, piece, :]), cast(h2b[:, n0:n0 + 512]),
                            start=True, stop=True, tile_position=(0, 32 * g))
                    nc.scalar.activation(e3b[:, ch * 512:(ch + 1) * 512], ps3[:],
                                         AF.Exp, scale=-2.0,
                                         bias=b3b[:, piece:piece + 1])

            def dve_combine(e, s, last_step):
                # k = tanh = 2*r - 1 with r = 1/(1+exp(-2x)); dt = 1
                sl = slice(s * 1024, (s + 1) * 1024)
                ts, tt = nc.vector.tensor_scalar, nc.vector.tensor_tensor
                add, sub, mult = (mybir.AluOpType.add, mybir.AluOpType.subtract,
                                  mybir.AluOpType.mult)
                ts(e3b[:, sl], e3b[:, sl], 1.0, None, add)       # u = e + 1
                nc.vector.reciprocal(e3b[:, sl], e3b[:, sl])     # r = 1/u
                r, t0 = e3b[:, sl], t0b[:, sl]
                if e == 0:
                    ts(acc[:, sl], r, 2.0, -1.0, mult, add)      # acc = k1
                    ts(t0, r, 0.5, None, sub)                    # k1/2 = r - 1/2
                    tt(zin[:, sl], z[:, sl], t0, add)            # zmid = z + k1/2
                elif e == 1:
                    ts(t0, r, 0.5, None, sub)                    # k2/2
                    tt(zin[:, sl], z[:, sl], t0, add)            # zmid = z + k2/2
                    ts(t0, t0, 4.0, None, mult)                  # 2*k2
                    tt(acc[:, sl], acc[:, sl], t0, add)
                elif e == 2:
                    ts(t0, r, 2.0, -1.0, mult, add)              # k3
                    tt(zin[:, sl], z[:, sl], t0, add)            # zmid = z + k3
                    ts(t0, t0, 2.0, None, mult)                  # 2*k3
                    tt(acc[:, sl], acc[:, sl], t0, add)
                else:
                    ts(t0, r, 2.0, -1.0, mult, add)              # k4
                    tt(acc[:, sl], acc[:, sl], t0, add)
                    ts(t0, acc[:, sl], 1.0 / 6.0, None, mult)
                    tt(z[:, sl], z[:, sl], t0, add)              # z += acc/6
                    if not last_step:
                        nc.vector.tensor_copy(zin[:, sl], z[:, sl])

            imgs = [g * 4 + s for s in range(4) for g in range(4)]  # s-major
            n_evals = int(os.environ.get("ODE_NEVALS", "16"))
            for eidx in range(n_evals):
                step, e = eidx // 4, eidx % 4
                _, piece = _EVAL_TP[eidx]
                c2done = [0] * 4

                def after_c2(j):
                    s = j % 4
                    c2done[s] += 1
                    if c2done[s] == 4:
                        conv3(s, piece)
                        dve_combine(e, s, last_step=(step == STEPS - 1))

                for i, j in enumerate(imgs):
                    conv1(j, piece, eidx)
                    if i >= LAG:
                        after_c2_j = imgs[i - LAG]
                        conv2(after_c2_j, piece)
                        after_c2(after_c2_j)
                for j in imgs[-LAG:]:
                    conv2(j, piece)
                    after_c2(j)

        # ---- readout: logits, loss, accuracy ----
        with (
            tc.tile_pool(name="ro", bufs=1) as rop,
            tc.tile_pool(name="pro", bufs=1, space="PSUM") as prop,
        ):
            zR = rop.tile([128, 4096], F32)
            rob_w = rop.tile([128, 2560], F32)
            ohb = rop.tile([16, 10], F32)
            rbb = rop.tile([16, 10], F32)
            nc.sync.dma_start(rob_w[:], ro_d[:])
            nc.sync.dma_start(ohb[:], oh_d[:])
            nc.sync.dma_start(rbb[:], rb_d[:])
            if debug:
                nc.sync.dma_start(zf_d[:], z[:])

            ro_mode = os.environ.get("ODE_RO", "full")
            # refold z[g*32+c, s*1024 + t*256 + q] -> zR[t*32+c, (g*4+s)*256 + q]
            zv = z.rearrange("p (s t q) -> p s t q", t=4, q=256)
            zRv = zR.rearrange("p (g s q) -> p g s q", s=4, q=256)
            for g in range(4):
                for t in range(4):
                    nc.sync.dma_start(zRv[32 * t:32 * t + 32, g, :, :],
                                      zv[32 * g:32 * g + 32, :, t, :])

            lt = rop.tile([32, 32], F32)
            ltT = rop.tile([32, 32], F32)
            nc.vector.memset(lt[:], 0.0)
            if ro_mode in ("full", "nostat"):
                lg_ps = prop.tile([10, 16], F32)
                zRq = zR.rearrange("p (j q) -> p j q", q=256)
                for q in range(256):
                    nc.tensor.matmul(lg_ps[:, :], rob_w[:, 10 * q:10 * q + 10],
                                     zRq[:, :, q], start=(q == 0), stop=(q == 255))
                nc.scalar.activation(lt[0:10, 0:16], lg_ps[:, :], AF.Identity,
                                     bias=0.0)
            nc.vector.transpose(ltT[:], lt[:])

            lgt = rop.tile([16, 10], F32)
            nc.vector.tensor_tensor(lgt[:], ltT[0:16, 0:10], rbb[:],
                                    mybir.AluOpType.add)
            if debug:
                nc.sync.dma_start(lg_d[:], lgt[:])

            if ro_mode in ("nostat", "nomm", "none"):
                sm0 = rop.tile([2, 1], F32)
                nc.vector.memset(sm0[:], 0.0)
                nc.sync.dma_start(out_d[:], sm0[:])
            else:
                mx = rop.tile([16, 1], F32)
                nc.vector.tensor_reduce(mx[:], lgt[:], mybir.AxisListType.X,
                                        mybir.AluOpType.max)
                sx = rop.tile([16, 10], F32)
                nc.vector.tensor_scalar(sx[:], lgt[:], mx[:], None,
                                        mybir.AluOpType.subtract)
                ex = rop.tile([16, 10], F32)
                nc.scalar.activation(ex[:], sx[:], AF.Exp)
                se = rop.tile([16, 1], F32)
                nc.vector.tensor_reduce(se[:], ex[:], mybir.AxisListType.X,
                                        mybir.AluOpType.add)
                lse = rop.tile([16, 1], F32)
                nc.scalar.activation(lse[:], se[:], AF.Ln)

                prod = rop.tile([16, 10], F32)
                tcorr = rop.tile([16, 1], F32)
                nc.vector.tensor_tensor(prod[:], lgt[:], ohb[:],
                                        mybir.AluOpType.mult)
                nc.vector.tensor_reduce(tcorr[:], prod[:], mybir.AxisListType.X,
                                        mybir.AluOpType.add)

                lossv = rop.tile([16, 1], F32)
                accv = rop.tile([16, 1], F32)
                nc.vector.tensor_tensor(lossv[:], lse[:], mx[:],
                                        mybir.AluOpType.add)
                nc.vector.tensor_tensor(lossv[:], lossv[:], tcorr[:],
                                        mybir.AluOpType.subtract)
                nc.vector.tensor_tensor(accv[:], mx[:], tcorr[:],
                                        mybir.AluOpType.is_equal)

                lv2 = rop.tile([128, 2], F32)
                nc.vector.memset(lv2[:], 0.0)
                nc.vector.tensor_copy(lv2[0:16, 0:1], lossv[:])
                nc.vector.tensor_copy(lv2[0:16, 1:2], accv[:])
                ones = rop.tile([128, 1], F32)
                nc.vector.memset(ones[:], 1.0)
                sm_ps = prop.tile([2, 1], F32)
                nc.tensor.matmul(sm_ps[:, :], lv2[:], ones[:],
                                 start=True, stop=True)
                sm = rop.tile([2, 1], F32)
                nc.scalar.activation(sm[:], sm_ps[:, :], AF.Identity, bias=0.0)
                nc.sync.dma_start(out_d[:], sm[:])

    nc.compile()
    return nc


# ---------------- host-side input prep ----------------

def prep_in_maps(inputs):
    DT_np = {"bf16": ml_dtypes.bfloat16, "fp32r": np.float32,
             "fp32": np.float32}[DT_NAME]
    f = np.float32
    x = np.asarray(inputs["x"], f)          # [128, 3, 32, 32]
    y = np.asarray(inputs["y"]).astype(np.int64)  # [128]
    aug_W = np.asarray(inputs["aug_W"], f)  # [32, 3]
    aug_b = np.asarray(inputs["aug_b"], f)  # [32]
    W1 = np.asarray(inputs["W1"], f)        # [4, 128, 33]
    b1 = np.asarray(inputs["b1"], f)        # [4, 128]
    W2 = np.asarray(inputs["W2"], f)        # [4, 128, 128, 3, 3]
    b2 = np.asarray(inputs["b2"], f)        # [4, 128]
    W3 = np.asarray(inputs["W3"], f)        # [4, 32, 128]
    b3 = np.asarray(inputs["b3"], f)        # [4, 32]
    ro_W = np.asarray(inputs["ro_W"], f)    # [10, 32768]
    ro_b = np.asarray(inputs["ro_b"], f)    # [10]

    # xf: [core, 128 = g*32+c (c<3), 4096 = s*1024 + pos]
    xr = x.reshape(NCORES, 4, 4, 3, 1024)          # [core, g, s, c, pos]
    xf = np.zeros((NCORES, 4, 32, 4, 1024), f)     # [core, g, c, s, pos]
    xf[:, :, :3] = xr.transpose(0, 1, 3, 2, 4)
    xf = np.ascontiguousarray(xf.reshape(NCORES, 128, 4096)).astype(DT_np)

    # w1s[p, g, 32g+c, m] = W1[p, m, 1+c]; zero outside group g's strip
    w1T = W1[:, :, 1:].transpose(0, 2, 1)          # [p, c, m]
    w1s = np.zeros((4, 4, 128, 128), f)
    for g in range(4):
        w1s[:, g, 32 * g:32 * g + 32, :] = w1T
    w1s = w1s.astype(DT_np)
    w2s = np.ascontiguousarray(
        W2.transpose(0, 3, 4, 2, 1).reshape(4, 9, 128, 128)).astype(DT_np)
    w3s = np.ascontiguousarray(W3.transpose(0, 2, 1)).astype(DT_np)
    # augw[g, 32g+i, m] = aug_W[m, i] (i<3); zero elsewhere
    augw = np.zeros((4, 128, 32), f)
    for g in range(4):
        augw[g, 32 * g:32 * g + 3, :] = aug_W.T
    augw = augw.astype(DT_np)

    b1e = np.empty((128, 16), f)
    for eidx, (t, piece) in enumerate(_EVAL_TP):
        b1e[:, eidx] = b1[piece] + np.float32(t) * W1[piece][:, 0]
    b2s = np.ascontiguousarray(b2.T)                       # [128, 4]
    b3s = np.ascontiguousarray(-2.0 * np.tile(b3, (1, 4)).T)  # [128, 4] folded
    augb = np.tile(aug_b, 4)[:, None].astype(f)            # [128, 1]

    # row: [128 = t*32+c, q*10+cls] = ro_W[cls, c*1024 + t*256 + q]
    ro4 = ro_W.reshape(10, 32, 4, 256)                     # [cls, c, t, q]
    row = np.ascontiguousarray(
        ro4.transpose(2, 1, 3, 0).reshape(128, 2560))      # [t*32+c, q, cls]
    rob = np.tile(ro_b, (16, 1)).astype(f)                 # [16, 10]

    eye = np.eye(10, dtype=f)
    in_maps = []
    for k in range(NCORES):
        oneh = eye[y[k * BL:(k + 1) * BL]]
        in_maps.append({
            "xf": xf[k], "w1s": w1s, "w2s": w2s, "w3s": w3s, "augw": augw,
            "b1e": b1e, "b2s": b2s, "b3s": b3s, "augb": augb,
            "row": row, "oneh": oneh, "rob": rob,
        })
    return in_maps


_NC_CACHE = {}


def _get_nc(debug=False):
    key = (DT_NAME, debug)
    if key not in _NC_CACHE:
        _NC_CACHE[key] = build_nc(debug)
    return _NC_CACHE[key]


def run(inputs, debug=False, **spmd_kwargs):
    nc = _get_nc(debug)
    in_maps = prep_in_maps(inputs)
    res = run_bass_kernel_spmd(nc, in_maps, core_ids=list(range(NCORES)),
                               **spmd_kwargs)
    loss = sum(r["outv"][0, 0] for r in res.results) / 128.0
    accu = sum(r["outv"][1, 0] for r in res.results)
    out = (np.asarray(loss, np.float32), np.asarray(accu, np.float32))
    return out, res


def kernel(**inputs):
    out, _ = run(inputs)
    return out



# revision 2
# speedup vs baseline: 1.5483x; 1.5483x over previous
"""Trainium2 Bass kernel for nn_NeuralODECNN (RK4 neural-ODE CNN forward).

Self-contained: hardcodes all shapes. Data-parallel over batch across 8
NeuronCores (16 images per core); all params replicated.

Per-core on-chip layouts (B_local=16 images, j = g*4 + s, g=group, s=slot;
pos(j) = s*4 + g so that the s-major eval order touches adjacent slots):
  folded  [128 = g*32 + c, 4096 = s*1024 + y*32 + x]   (z / k / x tensors, c<32)
  wide    [128 = channel,  16384 = pos*1024 + y*32 + x]  (h2, 128 channels)
  h1pad   [128, 16(pos), 34, 34]  zero-padded per image for the 3x3 conv taps

conv1 (32ch -> 128ch 1x1): row-tiled K=32 matmuls (row group = image group)
conv2 (3x3 SAME): 9 accumulating matmuls per output chunk, shifted AP taps
conv3 (128ch -> 32ch 1x1): col-tiled M=32 matmuls writing folded psum
softplus(x) = Ln(Exp(x)+1) on ScalarE; the Ln pass is batched over image
pairs ([128,2048]) to amortize ACT instruction overhead.
All ACT functions (Exp/Ln/Identity) live in the natural_log_exp_and_others
table set; get_activation_tables is patched (order-preserving membership
strip) so walrus picks that single set instead of ping-ponging between
exp_and_others and natural_log (which cost ~900us in table reloads).
tanh(x)     = 2*recip(1+Exp(-2x)) - 1 (Exp on ScalarE, recip on VectorE)
t-channel of conv1 folded into per-eval biases (b1e) on the host.
readout: DMA refold z -> [128 = t*32+c, 16*256], 256 accumulating fp32
matmuls -> logits psum [10,16]; log-softmax + onehot loss + argmax-accuracy
on device; host sums the 8 per-core (loss_sum, acc_sum) pairs.
"""

import functools
import os
from contextlib import ExitStack

import ml_dtypes
import numpy as np

import concourse.bacc as bacc
import concourse.hw_specs as hw_specs
import concourse.mybir as mybir
import concourse.tile as tile
from concourse.bass_utils import run_bass_kernel_spmd

F32 = mybir.dt.float32
AF = mybir.ActivationFunctionType

NCORES = 8
BL = 16          # images per core
STEPS = 4        # RK4 steps (= pieces, STEPS_PER_PIECE=1, dt=1)
DT_NAME = os.environ.get("ODE_DT", "bf16")   # bf16 | fp32r | fp32
LAG = 2          # conv2 image lag behind conv1 within an eval

# eval schedule: step i evals use t = i + {0,.5,.5,1}, piece = i (k1..k3) or
# min(i+1,3) (k4, since floor(i+1) indexes the next piece)
_EVAL_TP = [(i + dt, i if k < 3 else min(i + 1, 3))
            for i in range(4) for k, dt in enumerate((0.0, 0.5, 0.5, 1.0))]


def _pos(j):
    """SBUF image slot for image j=g*4+s: s-major so the eval order
    (s fixed, g ascending) writes adjacent slots -> batched Ln APs."""
    return (j % 4) * 4 + j // 4


def _patch_act_tables():
    """Make walrus resolve every ACT function to the one table set that
    contains all of {Exp, Ln, Identity}: strip those functions from the
    sets that precede natural_log_exp_and_others in act_info.json order.
    Order (and therefore act_func_set_id indices) is preserved; only the
    first-match selection inside bass's insert_act_table_loads changes.
    Without this, alternating Exp/Ln emits an ACT_TABLE_LOAD (~1.3us)
    per transition: 706 loads / ~900us on the baseline trace."""
    orig = hw_specs.get_activation_tables

    @functools.cache
    def gat(arch):
        t = dict(orig(arch))
        pref = "natural_log_exp_and_others"
        if pref not in t:
            return t
        keep = t[pref]
        return {k: (v if k == pref else v - keep) for k, v in t.items()}

    bacc.get_activation_tables = gat


_patch_act_tables()


def _mm_dtype():
    return {"bf16": mybir.dt.bfloat16, "fp32r": F32, "fp32": F32}[DT_NAME]


def build_nc(debug=False):
    DT = _mm_dtype()
    if DT_NAME == "fp32r":
        cast = lambda ap: ap.bitcast(mybir.dt.float32r)  # noqa: E731
    else:
        cast = lambda ap: ap  # noqa: E731

    nc = bacc.Bacc("TRN2")

    xf_d = nc.dram_tensor("xf", [128, 4096], DT, kind="ExternalInput")
    # conv1/aug weights are zero-padded to full K=128 (rows outside the
    # group's 32-partition strip are zero) — row-tiled partial-K matmuls
    # return garbage on this HW path, full-K costs the same N cycles.
    w1_d = nc.dram_tensor("w1s", [4, 4, 128, 128], DT, kind="ExternalInput")
    w2_d = nc.dram_tensor("w2s", [4, 9, 128, 128], DT, kind="ExternalInput")
    w3_d = nc.dram_tensor("w3s", [4, 128, 32], DT, kind="ExternalInput")
    aw_d = nc.dram_tensor("augw", [4, 128, 32], DT, kind="ExternalInput")
    b1_d = nc.dram_tensor("b1e", [128, 16], F32, kind="ExternalInput")
    b2_d = nc.dram_tensor("b2s", [128, 4], F32, kind="ExternalInput")
    b3_d = nc.dram_tensor("b3s", [128, 4], F32, kind="ExternalInput")  # -2*b3 folded
    ab_d = nc.dram_tensor("augb", [128, 1], F32, kind="ExternalInput")
    ro_d = nc.dram_tensor("row", [128, 2560], F32, kind="ExternalInput")
    oh_d = nc.dram_tensor("oneh", [16, 10], F32, kind="ExternalInput")
    rb_d = nc.dram_tensor("rob", [16, 10], F32, kind="ExternalInput")
    out_d = nc.dram_tensor("outv", [2, 1], F32, kind="ExternalOutput")
    if debug:
        zf_d = nc.dram_tensor("zf", [128, 4096], F32, kind="ExternalOutput")
        lg_d = nc.dram_tensor("lg", [16, 10], F32, kind="ExternalOutput")

    with tile.TileContext(nc) as tc, ExitStack() as ctx:
        sing = ctx.enter_context(tc.tile_pool(name="sing", bufs=1))
        z = sing.tile([128, 4096], F32)
        zin = sing.tile([128, 4096], DT)
        acc = sing.tile([128, 4096], F32)
        w1b = sing.tile([128, 4, 4, 128], DT)
        w2b = sing.tile([128, 4, 9, 128], DT)
        w3b = sing.tile([128, 4, 32], DT)
        awb = sing.tile([128, 4, 32], DT)
        b1b = sing.tile([128, 16], F32)
        b2b = sing.tile([128, 4], F32)
        b3b = sing.tile([128, 4], F32)
        abb = sing.tile([128, 1], F32)

        nc.sync.dma_start(w1b[:], w1_d.rearrange("p g i m -> i p g m"))
        nc.sync.dma_start(w2b[:], w2_d.rearrange("p t i m -> i p t m"))
        nc.sync.dma_start(w3b[:], w3_d.rearrange("p i m -> i p m"))
        nc.sync.dma_start(awb[:], aw_d.rearrange("g i m -> i g m"))
        nc.sync.dma_start(b1b[:], b1_d[:])
        nc.sync.dma_start(b2b[:], b2_d[:])
        nc.sync.dma_start(b3b[:], b3_d[:])
        nc.sync.dma_start(abb[:], ab_d[:])

        with (
            tc.tile_pool(name="mid", bufs=1) as mid,
            tc.tile_pool(name="p1", bufs=1, space="PSUM") as p1p,
            tc.tile_pool(name="p2", bufs=2, space="PSUM") as p2p,
            tc.tile_pool(name="p3", bufs=2, space="PSUM") as p3p,
            tc.tile_pool(name="stg", bufs=2) as stg,
        ):
            h1pad = mid.tile([128, 16, 34, 34], DT)
            h2b = mid.tile([128, 16384], DT)
            e3b = mid.tile([128, 4096], F32)
            t0b = mid.tile([128, 4096], F32)
            xfb = mid.tile([128, 4096], DT)

            nc.sync.dma_start(xfb[:], xf_d[:])
            nc.vector.memset(h1pad[:], 0.0)

            # ---- augment: z0 = aug_W @ x + aug_b (col-tiled, zero-pad K) ----
            for s in range(4):
                ps = p1p.tile([128, 1024], F32, tag="ps1")
                for g in range(4):
                    for h in range(2):
                        n0 = s * 1024 + h * 512
                        nc.tensor.matmul(
                            ps[32 * g:32 * g + 32, h * 512:(h + 1) * 512],
                            cast(awb[:, g, :]),
                            cast(xfb[:, n0:n0 + 512]),
                            start=True, stop=True, tile_position=(0, 32 * g))
                sl = slice(s * 1024, (s + 1) * 1024)
                nc.scalar.activation(z[:, sl], ps[:], AF.Identity, bias=abb[:, 0:1])
                nc.vector.tensor_copy(zin[:, sl], z[:, sl])

            # ---- the 16 RK4 sub-evaluations ----
            def conv1(j, piece, eidx, st2, half, pair0):
                g, s = j // 4, j % 4
                ps1 = p1p.tile([128, 1024], F32, tag="ps1")
                for h in range(2):
                    n0 = s * 1024 + h * 512
                    nc.tensor.matmul(
                        ps1[:, h * 512:(h + 1) * 512],
                        cast(w1b[:, piece, g, :]),
                        cast(zin[:, n0:n0 + 512]),
                        start=True, stop=True)
                nc.scalar.activation(st2[:, half * 1024:(half + 1) * 1024],
                                     ps1[:], AF.Exp, bias=b1b[:, eidx:eidx + 1])
                if half == 1:
                    p0 = _pos(pair0)
                    nc.scalar.activation(
                        h1pad[:, p0:p0 + 2, 1:33, 1:33],
                        st2.rearrange("p (j a b) -> p j a b", a=32, b=32),
                        AF.Ln, bias=1.0)

            def conv2(j, piece, st2c, half, pair0):
                ps2 = p2p.tile([128, 1024], F32, tag="ps2")
                pj = _pos(j)
                for tap in range(9):
                    dy, dx = tap // 3, tap % 3
                    for h in range(2):
                        y0 = h * 16 + dy
                        nc.tensor.matmul(
                            ps2[:, h * 512:(h + 1) * 512],
                            cast(w2b[:, piece, tap, :]),
                            cast(h1pad[:, pj, y0:y0 + 16, dx:dx + 32]),
                            start=(tap == 0), stop=(tap == 8))
                nc.scalar.activation(st2c[:, half * 1024:(half + 1) * 1024],
                                     ps2[:], AF.Exp, bias=b2b[:, piece:piece + 1])
                if half == 1:
                    p0 = _pos(pair0)
                    nc.scalar.activation(h2b[:, p0 * 1024:(p0 + 2) * 1024],
                                         st2c[:], AF.Ln, bias=1.0)

            def conv3(s, piece):
                for ch in (2 * s, 2 * s + 1):
                    ps3 = p3p.tile([128, 512], F32, tag="ps3")
                    for g in range(4):
                        n0 = (s * 4 + g) * 1024 + (ch % 2) * 512
                        nc.tensor.matmul(
                            ps3[32 * g:32 * g + 32, :],
                            cast(w3b[:, piece, :]), cast(h2b[:, n0:n0 + 512]),
                            start=True, stop=True, tile_position=(0, 32 * g))
                    nc.scalar.activation(e3b[:, ch * 512:(ch + 1) * 512], ps3[:],
                                         AF.Exp, scale=-2.0,
                                         bias=b3b[:, piece:piece + 1])

            def dve_combine(e, s, last_step):
                # k = tanh = 2*r - 1 with r = 1/(1+exp(-2x)); dt = 1
                sl = slice(s * 1024, (s + 1) * 1024)
                ts, tt = nc.vector.tensor_scalar, nc.vector.tensor_tensor
                add, sub, mult = (mybir.AluOpType.add, mybir.AluOpType.subtract,
                                  mybir.AluOpType.mult)
                ts(e3b[:, sl], e3b[:, sl], 1.0, None, add)       # u = e + 1
                nc.vector.reciprocal(e3b[:, sl], e3b[:, sl])     # r = 1/u
                r, t0 = e3b[:, sl], t0b[:, sl]
                if e == 0:
                    ts(acc[:, sl], r, 2.0, -1.0, mult, add)      # acc = k1
                    ts(t0, r, 0.5, None, sub)                    # k1/2 = r - 1/2
                    tt(zin[:, sl], z[:, sl], t0, add)            # zmid = z + k1/2
                elif e == 1:
                    ts(t0, r, 0.5, None, sub)                    # k2/2
                    tt(zin[:, sl], z[:, sl], t0, add)            # zmid = z + k2/2
                    ts(t0, t0, 4.0, None, mult)                  # 2*k2
                    tt(acc[:, sl], acc[:, sl], t0, add)
                elif e == 2:
                    ts(t0, r, 2.0, -1.0, mult, add)              # k3
                    tt(zin[:, sl], z[:, sl], t0, add)            # zmid = z + k3
                    ts(t0, t0, 2.0, None, mult)                  # 2*k3
                    tt(acc[:, sl], acc[:, sl], t0, add)
                else:
                    ts(t0, r, 2.0, -1.0, mult, add)              # k4
                    tt(acc[:, sl], acc[:, sl], t0, add)
                    ts(t0, acc[:, sl], 1.0 / 6.0, None, mult)
                    tt(z[:, sl], z[:, sl], t0, add)              # z += acc/6
                    if not last_step:
                        nc.vector.tensor_copy(zin[:, sl], z[:, sl])

            imgs = [g * 4 + s for s in range(4) for g in range(4)]  # s-major
            n_evals = int(os.environ.get("ODE_NEVALS", "16"))
            for eidx in range(n_evals):
                step, e = eidx // 4, eidx % 4
                _, piece = _EVAL_TP[eidx]
                c2done = [0] * 4

                def after_c2(j):
                    s = j % 4
                    c2done[s] += 1
                    if c2done[s] == 4:
                        conv3(s, piece)
                        dve_combine(e, s, last_step=(step == STEPS - 1))

                st1 = st2c = None
                for i, j in enumerate(imgs):
                    if i % 2 == 0:
                        st1 = stg.tile([128, 2048], F32, tag="st")
                    conv1(j, piece, eidx, st1, i % 2, imgs[i - i % 2])
                    if i >= LAG:
                        i2 = i - LAG
                        if i2 % 2 == 0:
                            st2c = stg.tile([128, 2048], F32, tag="st2")
                        conv2(imgs[i2], piece, st2c, i2 % 2, imgs[i2 - i2 % 2])
                        after_c2(imgs[i2])
                for i2 in range(16 - LAG, 16):
                    if i2 % 2 == 0:
                        st2c = stg.tile([128, 2048], F32, tag="st2")
                    conv2(imgs[i2], piece, st2c, i2 % 2, imgs[i2 - i2 % 2])
                    after_c2(imgs[i2])

        # ---- readout: logits, loss, accuracy ----
        with (
            tc.tile_pool(name="ro", bufs=1) as rop,
            tc.tile_pool(name="pro", bufs=1, space="PSUM") as prop,
        ):
            zR = rop.tile([128, 4096], F32)
            rob_w = rop.tile([128, 2560], F32)
            ohb = rop.tile([16, 10], F32)
            rbb = rop.tile([16, 10], F32)
            nc.sync.dma_start(rob_w[:], ro_d[:])
            nc.sync.dma_start(ohb[:], oh_d[:])
            nc.sync.dma_start(rbb[:], rb_d[:])
            if debug:
                nc.sync.dma_start(zf_d[:], z[:])

            ro_mode = os.environ.get("ODE_RO", "full")
            # refold z[g*32+c, s*1024 + t*256 + q] -> zR[t*32+c, (g*4+s)*256 + q]
            zv = z.rearrange("p (s t q) -> p s t q", t=4, q=256)
            zRv = zR.rearrange("p (g s q) -> p g s q", s=4, q=256)
            for g in range(4):
                for t in range(4):
                    nc.sync.dma_start(zRv[32 * t:32 * t + 32, g, :, :],
                                      zv[32 * g:32 * g + 32, :, t, :])

            lt = rop.tile([32, 32], F32)
            ltT = rop.tile([32, 32], F32)
            nc.vector.memset(lt[:], 0.0)
            if ro_mode in ("full", "nostat"):
                lg_ps = prop.tile([10, 16], F32)
                zRq = zR.rearrange("p (j q) -> p j q", q=256)
                for q in range(256):
                    nc.tensor.matmul(lg_ps[:, :], rob_w[:, 10 * q:10 * q + 10],
                                     zRq[:, :, q], start=(q == 0), stop=(q == 255))
                nc.scalar.activation(lt[0:10, 0:16], lg_ps[:, :], AF.Identity,
                                     bias=0.0)
            nc.vector.transpose(ltT[:], lt[:])

            lgt = rop.tile([16, 10], F32)
            nc.vector.tensor_tensor(lgt[:], ltT[0:16, 0:10], rbb[:],
                                    mybir.AluOpType.add)
            if debug:
                nc.sync.dma_start(lg_d[:], lgt[:])

            if ro_mode in ("nostat", "nomm", "none"):
                sm0 = rop.tile([2, 1], F32)
                nc.vector.memset(sm0[:], 0.0)
                nc.sync.dma_start(out_d[:], sm0[:])
            else:
                mx = rop.tile([16, 1], F32)
                nc.vector.tensor_reduce(mx[:], lgt[:], mybir.AxisListType.X,
                                        mybir.AluOpType.max)
                sx = rop.tile([16, 10], F32)
                nc.vector.tensor_scalar(sx[:], lgt[:], mx[:], None,
                                        mybir.AluOpType.subtract)
                ex = rop.tile([16, 10], F32)
                nc.scalar.activation(ex[:], sx[:], AF.Exp)
                se = rop.tile([16, 1], F32)
                nc.vector.tensor_reduce(se[:], ex[:], mybir.AxisListType.X,
                                        mybir.AluOpType.add)
                lse = rop.tile([16, 1], F32)
                nc.scalar.activation(lse[:], se[:], AF.Ln)

                prod = rop.tile([16, 10], F32)
                tcorr = rop.tile([16, 1], F32)
                nc.vector.tensor_tensor(prod[:], lgt[:], ohb[:],
                                        mybir.AluOpType.mult)
                nc.vector.tensor_reduce(tcorr[:], prod[:], mybir.AxisListType.X,
                                        mybir.AluOpType.add)

                lossv = rop.tile([16, 1], F32)
                accv = rop.tile([16, 1], F32)
                nc.vector.tensor_tensor(lossv[:], lse[:], mx[:],
                                        mybir.AluOpType.add)
                nc.vector.tensor_tensor(lossv[:], lossv[:], tcorr[:],
                                        mybir.AluOpType.subtract)
                nc.vector.tensor_tensor(accv[:], mx[:], tcorr[:],
                                        mybir.AluOpType.is_equal)

                lv2 = rop.tile([128, 2], F32)
                nc.vector.memset(lv2[:], 0.0)
                nc.vector.tensor_copy(lv2[0:16, 0:1], lossv[:])
                nc.vector.tensor_copy(lv2[0:16, 1:2], accv[:])
                ones = rop.tile([128, 1], F32)
                nc.vector.memset(ones[:], 1.0)
                sm_ps = prop.tile([2, 1], F32)
                nc.tensor.matmul(sm_ps[:, :], lv2[:], ones[:],
                                 start=True, stop=True)
                sm = rop.tile([2, 1], F32)
                nc.scalar.activation(sm[:], sm_ps[:, :], AF.Identity, bias=0.0)
                nc.sync.dma_start(out_d[:], sm[:])

    nc.compile()
    return nc


# ---------------- host-side input prep ----------------

def prep_in_maps(inputs):
    DT_np = {"bf16": ml_dtypes.bfloat16, "fp32r": np.float32,
             "fp32": np.float32}[DT_NAME]
    f = np.float32
    x = np.asarray(inputs["x"], f)          # [128, 3, 32, 32]
    y = np.asarray(inputs["y"]).astype(np.int64)  # [128]
    aug_W = np.asarray(inputs["aug_W"], f)  # [32, 3]
    aug_b = np.asarray(inputs["aug_b"], f)  # [32]
    W1 = np.asarray(inputs["W1"], f)        # [4, 128, 33]
    b1 = np.asarray(inputs["b1"], f)        # [4, 128]
    W2 = np.asarray(inputs["W2"], f)        # [4, 128, 128, 3, 3]
    b2 = np.asarray(inputs["b2"], f)        # [4, 128]
    W3 = np.asarray(inputs["W3"], f)        # [4, 32, 128]
    b3 = np.asarray(inputs["b3"], f)        # [4, 32]
    ro_W = np.asarray(inputs["ro_W"], f)    # [10, 32768]
    ro_b = np.asarray(inputs["ro_b"], f)    # [10]

    # xf: [core, 128 = g*32+c (c<3), 4096 = s*1024 + pos]
    xr = x.reshape(NCORES, 4, 4, 3, 1024)          # [core, g, s, c, pos]
    xf = np.zeros((NCORES, 4, 32, 4, 1024), f)     # [core, g, c, s, pos]
    xf[:, :, :3] = xr.transpose(0, 1, 3, 2, 4)
    xf = np.ascontiguousarray(xf.reshape(NCORES, 128, 4096)).astype(DT_np)

    # w1s[p, g, 32g+c, m] = W1[p, m, 1+c]; zero outside group g's strip
    w1T = W1[:, :, 1:].transpose(0, 2, 1)          # [p, c, m]
    w1s = np.zeros((4, 4, 128, 128), f)
    for g in range(4):
        w1s[:, g, 32 * g:32 * g + 32, :] = w1T
    w1s = w1s.astype(DT_np)
    w2s = np.ascontiguousarray(
        W2.transpose(0, 3, 4, 2, 1).reshape(4, 9, 128, 128)).astype(DT_np)
    w3s = np.ascontiguousarray(W3.transpose(0, 2, 1)).astype(DT_np)
    # augw[g, 32g+i, m] = aug_W[m, i] (i<3); zero elsewhere
    augw = np.zeros((4, 128, 32), f)
    for g in range(4):
        augw[g, 32 * g:32 * g + 3, :] = aug_W.T
    augw = augw.astype(DT_np)

    b1e = np.empty((128, 16), f)
    for eidx, (t, piece) in enumerate(_EVAL_TP):
        b1e[:, eidx] = b1[piece] + np.float32(t) * W1[piece][:, 0]
    b2s = np.ascontiguousarray(b2.T)                       # [128, 4]
    b3s = np.ascontiguousarray(-2.0 * np.tile(b3, (1, 4)).T)  # [128, 4] folded
    augb = np.tile(aug_b, 4)[:, None].astype(f)            # [128, 1]

    # row: [128 = t*32+c, q*10+cls] = ro_W[cls, c*1024 + t*256 + q]
    ro4 = ro_W.reshape(10, 32, 4, 256)                     # [cls, c, t, q]
    row = np.ascontiguousarray(
        ro4.transpose(2, 1, 3, 0).reshape(128, 2560))      # [t*32+c, q, cls]
    rob = np.tile(ro_b, (16, 1)).astype(f)                 # [16, 10]

    eye = np.eye(10, dtype=f)
    in_maps = []
    for k in range(NCORES):
        oneh = eye[y[k * BL:(k + 1) * BL]]
        in_maps.append({
            "xf": xf[k], "w1s": w1s, "w2s": w2s, "w3s": w3s, "augw": augw,
            "b1e": b1e, "b2s": b2s, "b3s": b3s, "augb": augb,
            "row": row, "oneh": oneh, "rob": rob,
        })
    return in_maps


_NC_CACHE = {}


def _get_nc(debug=False):
    key = (DT_NAME, debug)
    if key not in _NC_CACHE:
        _NC_CACHE[key] = build_nc(debug)
    return _NC_CACHE[key]


def run(inputs, debug=False, **spmd_kwargs):
    nc = _get_nc(debug)
    in_maps = prep_in_maps(inputs)
    res = run_bass_kernel_spmd(nc, in_maps, core_ids=list(range(NCORES)),
                               **spmd_kwargs)
    loss = sum(r["outv"][0, 0] for r in res.results) / 128.0
    accu = sum(r["outv"][1, 0] for r in res.results)
    out = (np.asarray(loss, np.float32), np.asarray(accu, np.float32))
    return out, res


def kernel(**inputs):
    out, _ = run(inputs)
    return out


# revision 5
# speedup vs baseline: 1.7558x; 1.1340x over previous
"""Trainium2 Bass kernel for nn_NeuralODECNN (RK4 neural-ODE CNN forward).

Self-contained: hardcodes all shapes. Data-parallel over batch across 8
NeuronCores (16 images per core); all params replicated.

Per-core on-chip layouts (B_local=16 images, j = g*4 + s, g=group, s=slot;
pos(j) = s*4 + g = stream index, so the s-major eval order touches
adjacent slots and Ln passes batch over image pairs):
  folded  [128 = g*32 + c, 4096 = s*1024 + y*32 + x]   (z / k / x tensors, c<32)
  wide    [128 = channel,  16384 = pos*1024 + y*32 + x]  (h2, 128 channels)
  h1pad   [128, 16(pos), 34, 34]  zero-padded per image for the 3x3 conv taps

conv1 (32ch -> 128ch 1x1): row-tiled K=32 matmuls (row group = image group)
conv2 (3x3 SAME): 9 accumulating matmuls per output chunk, shifted AP taps
conv3 (128ch -> 32ch 1x1): col-tiled M=32 matmuls into one [128,1024] psum
softplus(x) = Ln(Exp(x)+1) on ScalarE; the Ln pass is batched over image
pairs ([128,2048]) to amortize ACT instruction overhead.
All ACT functions (Exp/Ln/Identity) live in the natural_log_exp_and_others
table set; get_activation_tables is patched (order-preserving membership
strip) so walrus picks that single set instead of ping-ponging between
exp_and_others and natural_log (which cost ~900us in table reloads).
tanh(x)     = 2*recip(1+Exp(-2x)) - 1 (Exp on ScalarE, recip on VectorE)
t-channel of conv1 folded into per-eval biases (b1e) on the host.

The 16 RK4 sub-evaluations run as ONE flat software-pipelined stream:
conv1 leads, conv2 lags LAG images, conv3+combine for a completed slot
are emitted C3LAG images later so their Ln/psum waits hide under conv2
matmuls of later images (and, across the eval boundary, under the next
eval's conv1 matmuls).

readout: z is refolded to zR [128 = t*32+c, 16*256] by per-slot DMAs
issued right after that slot's final combine (overlapping the last eval),
then 256 accumulating fp32 matmuls -> logits psum [10,16]; log-softmax +
onehot loss + argmax-accuracy on device; host sums the 8 per-core
(loss_sum, acc_sum) pairs.
"""

import functools
import os
from contextlib import ExitStack

import ml_dtypes
import numpy as np

import concourse.bacc as bacc
import concourse.hw_specs as hw_specs
import concourse.mybir as mybir
import concourse.tile as tile
from concourse.bass_utils import run_bass_kernel_spmd

F32 = mybir.dt.float32
AF = mybir.ActivationFunctionType

NCORES = 8
BL = 16          # images per core
STEPS = 4        # RK4 steps (= pieces, STEPS_PER_PIECE=1, dt=1)
DT_NAME = os.environ.get("ODE_DT", "bf16")   # bf16 | fp32r | fp32
LAG = 2          # conv2 image lag behind conv1 within the flat stream
C3LAG = 2        # conv3+combine emission lag behind slot completion

# eval schedule: step i evals use t = i + {0,.5,.5,1}, piece = i (k1..k3) or
# min(i+1,3) (k4, since floor(i+1) indexes the next piece)
_EVAL_TP = [(i + dt, i if k < 3 else min(i + 1, 3))
            for i in range(4) for k, dt in enumerate((0.0, 0.5, 0.5, 1.0))]


def _patch_act_tables():
    """Make walrus resolve every ACT function to the one table set that
    contains all of {Exp, Ln, Identity}: strip those functions from the
    other sets so first-match lands on natural_log_exp_and_others.
    Order (and therefore act_func_set_id indices) is preserved; only the
    selection inside bass's insert_act_table_loads changes."""
    orig = hw_specs.get_activation_tables

    @functools.cache
    def gat(arch):
        t = dict(orig(arch))
        pref = "natural_log_exp_and_others"
        if pref not in t:
            return t
        keep = t[pref]
        return {k: (v if k == pref else v - keep) for k, v in t.items()}

    bacc.get_activation_tables = gat


_patch_act_tables()


def _mm_dtype():
    return {"bf16": mybir.dt.bfloat16, "fp32r": F32, "fp32": F32}[DT_NAME]


def build_nc(debug=False):
    DT = _mm_dtype()
    if DT_NAME == "fp32r":
        cast = lambda ap: ap.bitcast(mybir.dt.float32r)  # noqa: E731
    else:
        cast = lambda ap: ap  # noqa: E731

    nc = bacc.Bacc("TRN2")

    xf_d = nc.dram_tensor("xf", [128, 4096], DT, kind="ExternalInput")
    # conv1/aug weights are zero-padded to full K=128 (rows outside the
    # group's 32-partition strip are zero) — row-tiled partial-K matmuls
    # return garbage on this HW path, full-K costs the same N cycles.
    w1_d = nc.dram_tensor("w1s", [4, 4, 128, 128], DT, kind="ExternalInput")
    w2_d = nc.dram_tensor("w2s", [4, 9, 128, 128], DT, kind="ExternalInput")
    w3_d = nc.dram_tensor("w3s", [4, 128, 32], DT, kind="ExternalInput")
    aw_d = nc.dram_tensor("augw", [4, 128, 32], DT, kind="ExternalInput")
    b1_d = nc.dram_tensor("b1e", [128, 16], F32, kind="ExternalInput")
    b2_d = nc.dram_tensor("b2s", [128, 4], F32, kind="ExternalInput")
    b3_d = nc.dram_tensor("b3s", [128, 4], F32, kind="ExternalInput")  # -2*b3 folded
    ab_d = nc.dram_tensor("augb", [128, 1], F32, kind="ExternalInput")
    ro_d = nc.dram_tensor("row", [128, 2560], F32, kind="ExternalInput")
    oh_d = nc.dram_tensor("oneh", [16, 10], F32, kind="ExternalInput")
    rb_d = nc.dram_tensor("rob", [16, 10], F32, kind="ExternalInput")
    out_d = nc.dram_tensor("outv", [2, 1], F32, kind="ExternalOutput")
    if debug:
        zf_d = nc.dram_tensor("zf", [128, 4096], F32, kind="ExternalOutput")
        lg_d = nc.dram_tensor("lg", [16, 10], F32, kind="ExternalOutput")

    with tile.TileContext(nc) as tc, ExitStack() as ctx:
        sing = ctx.enter_context(tc.tile_pool(name="sing", bufs=1))
        z = sing.tile([128, 4096], F32)
        zin = sing.tile([128, 4096], DT)
        acc = sing.tile([128, 4096], F32)
        w1b = sing.tile([128, 4, 4, 128], DT)
        w2b = sing.tile([128, 4, 9, 128], DT)
        w3b = sing.tile([128, 4, 32], DT)
        awb = sing.tile([128, 4, 32], DT)
        b1b = sing.tile([128, 16], F32)
        b2b = sing.tile([128, 4], F32)
        b3b = sing.tile([128, 4], F32)
        abb = sing.tile([128, 1], F32)
        zR = sing.tile([128, 4096], F32)
        rob_w = sing.tile([128, 2560], F32)
        ohb = sing.tile([16, 10], F32)
        rbb = sing.tile([16, 10], F32)

        nc.sync.dma_start(w1b[:], w1_d.rearrange("p g i m -> i p g m"))
        nc.sync.dma_start(w2b[:], w2_d.rearrange("p t i m -> i p t m"))
        nc.sync.dma_start(w3b[:], w3_d.rearrange("p i m -> i p m"))
        nc.sync.dma_start(awb[:], aw_d.rearrange("g i m -> i g m"))
        nc.sync.dma_start(b1b[:], b1_d[:])
        nc.sync.dma_start(b2b[:], b2_d[:])
        nc.sync.dma_start(b3b[:], b3_d[:])
        nc.sync.dma_start(abb[:], ab_d[:])
        nc.sync.dma_start(rob_w[:], ro_d[:])
        nc.sync.dma_start(ohb[:], oh_d[:])
        nc.sync.dma_start(rbb[:], rb_d[:])

        # refold view: zR[t*32+c, (g*4+s)*256 + q] = z[g*32+c, s*1024+t*256+q]
        zv = z.rearrange("p (s t q) -> p s t q", t=4, q=256)
        zRv = zR.rearrange("p (g s q) -> p g s q", s=4, q=256)

        with (
            tc.tile_pool(name="mid", bufs=1) as mid,
            tc.tile_pool(name="p1", bufs=1, space="PSUM") as p1p,
            tc.tile_pool(name="p2", bufs=2, space="PSUM") as p2p,
            tc.tile_pool(name="p3", bufs=1, space="PSUM") as p3p,
            tc.tile_pool(name="stg", bufs=2) as stg,
        ):
            h1pad = mid.tile([128, 16, 34, 34], DT)
            h2b = mid.tile([128, 16384], DT)
            e3b = mid.tile([128, 1024], F32)
            t0b = mid.tile([128, 1024], F32)
            xfb = mid.tile([128, 4096], DT)

            nc.sync.dma_start(xfb[:], xf_d[:])
            # only the 1-px border needs to be zero; the interior is
            # rewritten by every eval's Ln pass
            nc.vector.memset(h1pad[:, :, 0, :], 0.0)
            nc.vector.memset(h1pad[:, :, 33, :], 0.0)
            nc.vector.memset(h1pad[:, :, 1:33, 0], 0.0)
            nc.vector.memset(h1pad[:, :, 1:33, 33], 0.0)

            # ---- augment: z0 = aug_W @ x + aug_b (col-tiled, zero-pad K) ----
            for s in range(4):
                ps = p1p.tile([128, 1024], F32, tag="ps1")
                for g in range(4):
                    for h in range(2):
                        n0 = s * 1024 + h * 512
                        nc.tensor.matmul(
                            ps[32 * g:32 * g + 32, h * 512:(h + 1) * 512],
                            cast(awb[:, g, :]),
                            cast(xfb[:, n0:n0 + 512]),
                            start=True, stop=True, tile_position=(0, 32 * g))
                sl = slice(s * 1024, (s + 1) * 1024)
                nc.scalar.activation(z[:, sl], ps[:], AF.Identity, bias=abb[:, 0:1])
                nc.vector.tensor_copy(zin[:, sl], z[:, sl])

            # ---- the 16 RK4 sub-evaluations, one flat pipelined stream ----
            def conv1(i, piece, eidx, st2):
                # image imgs[i] = (i%4)*4 + i//4; h1pad slot = i
                g, s = i % 4, i // 4
                ps1 = p1p.tile([128, 1024], F32, tag="ps1")
                for h in range(2):
                    n0 = s * 1024 + h * 512
                    nc.tensor.matmul(
                        ps1[:, h * 512:(h + 1) * 512],
                        cast(w1b[:, piece, g, :]),
                        cast(zin[:, n0:n0 + 512]),
                        start=True, stop=True)
                half = i % 2
                nc.scalar.activation(st2[:, half * 1024:(half + 1) * 1024],
                                     ps1[:], AF.Exp, bias=b1b[:, eidx:eidx + 1])
                if half == 1:
                    nc.scalar.activation(
                        h1pad[:, i - 1:i + 1, 1:33, 1:33],
                        st2.rearrange("p (j a b) -> p j a b", a=32, b=32),
                        AF.Ln, bias=1.0)

            def conv2(i, piece, st2c):
                ps2 = p2p.tile([128, 1024], F32, tag="ps2")
                for tap in range(9):
                    dy, dx = tap // 3, tap % 3
                    for h in range(2):
                        y0 = h * 16 + dy
                        nc.tensor.matmul(
                            ps2[:, h * 512:(h + 1) * 512],
                            cast(w2b[:, piece, tap, :]),
                            cast(h1pad[:, i, y0:y0 + 16, dx:dx + 32]),
                            start=(tap == 0), stop=(tap == 8))
                half = i % 2
                nc.scalar.activation(st2c[:, half * 1024:(half + 1) * 1024],
                                     ps2[:], AF.Exp, bias=b2b[:, piece:piece + 1])
                if half == 1:
                    nc.scalar.activation(h2b[:, (i - 1) * 1024:(i + 1) * 1024],
                                         st2c[:], AF.Ln, bias=1.0)

            def conv3(s, piece):
                # both 512-chunks of the slot into one [128,1024] psum tile
                ps3 = p3p.tile([128, 1024], F32, tag="ps3")
                for half in range(2):
                    for g in range(4):
                        n0 = (s * 4 + g) * 1024 + half * 512
                        nc.tensor.matmul(
                            ps3[32 * g:32 * g + 32,
                                half * 512:(half + 1) * 512],
                            cast(w3b[:, piece, :]), cast(h2b[:, n0:n0 + 512]),
                            start=True, stop=True, tile_position=(0, 32 * g))
                nc.scalar.activation(e3b[:], ps3[:], AF.Exp, scale=-2.0,
                                     bias=b3b[:, piece:piece + 1])

            def dve_combine(e, s, last_step):
                # k = tanh = 2*r - 1 with r = 1/(1+exp(-2x)); dt = 1
                sl = slice(s * 1024, (s + 1) * 1024)
                ts, tt = nc.vector.tensor_scalar, nc.vector.tensor_tensor
                add, sub, mult = (mybir.AluOpType.add, mybir.AluOpType.subtract,
                                  mybir.AluOpType.mult)
                ts(e3b[:], e3b[:], 1.0, None, add)               # u = e + 1
                # u in [1, ~5e8]: no 0/denorm/inf, so the ~51-ULP fast
                # reciprocal (5x faster than reciprocal()) is safe here
                nc.vector.reciprocal_approx_fast(e3b[:], e3b[:])  # r = 1/u
                r, t0 = e3b[:], t0b[:]
                if e == 0:
                    ts(acc[:, sl], r, 2.0, -1.0, mult, add)      # acc = k1
                    ts(t0, r, 0.5, None, sub)                    # k1/2 = r - 1/2
                    tt(zin[:, sl], z[:, sl], t0, add)            # zmid = z + k1/2
                elif e == 1:
                    ts(t0, r, 0.5, None, sub)                    # k2/2
                    tt(zin[:, sl], z[:, sl], t0, add)            # zmid = z + k2/2
                    ts(t0, t0, 4.0, None, mult)                  # 2*k2
                    tt(acc[:, sl], acc[:, sl], t0, add)
                elif e == 2:
                    ts(t0, r, 2.0, -1.0, mult, add)              # k3
                    tt(zin[:, sl], z[:, sl], t0, add)            # zmid = z + k3
                    ts(t0, t0, 2.0, None, mult)                  # 2*k3
                    tt(acc[:, sl], acc[:, sl], t0, add)
                else:
                    ts(t0, r, 2.0, -1.0, mult, add)              # k4
                    tt(acc[:, sl], acc[:, sl], t0, add)
                    ts(t0, acc[:, sl], 1.0 / 6.0, None, mult)
                    tt(z[:, sl], z[:, sl], t0, add)              # z += acc/6
                    if not last_step:
                        nc.vector.tensor_copy(zin[:, sl], z[:, sl])

            def refold_slot(s):
                for g in range(4):
                    for t in range(4):
                        nc.sync.dma_start(zRv[32 * t:32 * t + 32, g, s, :],
                                          zv[32 * g:32 * g + 32, s, t, :])

            n_evals = int(os.environ.get("ODE_NEVALS", "16"))
            total = n_evals * BL
            pending = {}
            c2done = [[0] * 4 for _ in range(n_evals)]
            refolded = [False] * 4

            def fire_c3(ev, s):
                step, e = ev // 4, ev % 4
                piece = _EVAL_TP[ev][1]
                conv3(s, piece)
                dve_combine(e, s, last_step=(step == STEPS - 1))
                if ev == n_evals - 1 and n_evals == 16:
                    refold_slot(s)
                    refolded[s] = True

            st1 = st2c = None
            for I in range(total + LAG):
                for ev, s in pending.pop(I, ()):
                    fire_c3(ev, s)
                if I < total:
                    ev1, i1 = divmod(I, BL)
                    if i1 % 2 == 0:
                        st1 = stg.tile([128, 2048], F32, tag="st")
                    conv1(i1, _EVAL_TP[ev1][1], ev1, st1)
                J = I - LAG
                if J >= 0:
                    ev2, i2 = divmod(J, BL)
                    if i2 % 2 == 0:
                        st2c = stg.tile([128, 2048], F32, tag="st2")
                    conv2(i2, _EVAL_TP[ev2][1], st2c)
                    s = i2 // 4
                    c2done[ev2][s] += 1
                    if c2done[ev2][s] == 4:
                        pending.setdefault(I + C3LAG, []).append((ev2, s))
            for I in range(total + LAG, total + LAG + C3LAG + 1):
                for ev, s in pending.pop(I, ()):
                    fire_c3(ev, s)
            assert not pending

            if not all(refolded):   # debug fallback (ODE_NEVALS < 16)
                for s in range(4):
                    if not refolded[s]:
                        refold_slot(s)

        # ---- readout: logits, loss, accuracy ----
        with (
            tc.tile_pool(name="ro", bufs=1) as rop,
            tc.tile_pool(name="pro", bufs=1, space="PSUM") as prop,
        ):
            if debug:
                nc.sync.dma_start(zf_d[:], z[:])

            ro_mode = os.environ.get("ODE_RO", "full")
            lt = rop.tile([32, 32], F32)
            ltT = rop.tile([32, 32], F32)
            nc.vector.memset(lt[:], 0.0)
            if ro_mode in ("full", "nostat"):
                lg_ps = prop.tile([10, 16], F32)
                zRq = zR.rearrange("p (j q) -> p j q", q=256)
                for q in range(256):
                    nc.tensor.matmul(lg_ps[:, :], rob_w[:, 10 * q:10 * q + 10],
                                     zRq[:, :, q], start=(q == 0), stop=(q == 255))
                nc.scalar.activation(lt[0:10, 0:16], lg_ps[:, :], AF.Identity,
                                     bias=0.0)
            nc.vector.transpose(ltT[:], lt[:])

            lgt = rop.tile([16, 10], F32)
            nc.vector.tensor_tensor(lgt[:], ltT[0:16, 0:10], rbb[:],
                                    mybir.AluOpType.add)
            if debug:
                nc.sync.dma_start(lg_d[:], lgt[:])

            if ro_mode in ("nostat", "nomm", "none"):
                sm0 = rop.tile([2, 1], F32)
                nc.vector.memset(sm0[:], 0.0)
                nc.sync.dma_start(out_d[:], sm0[:])
            else:
                mx = rop.tile([16, 1], F32)
                nc.vector.tensor_reduce(mx[:], lgt[:], mybir.AxisListType.X,
                                        mybir.AluOpType.max)
                sx = rop.tile([16, 10], F32)
                nc.vector.tensor_scalar(sx[:], lgt[:], mx[:], None,
                                        mybir.AluOpType.subtract)
                ex = rop.tile([16, 10], F32)
                nc.scalar.activation(ex[:], sx[:], AF.Exp)
                se = rop.tile([16, 1], F32)
                nc.vector.tensor_reduce(se[:], ex[:], mybir.AxisListType.X,
                                        mybir.AluOpType.add)
                lse = rop.tile([16, 1], F32)
                nc.scalar.activation(lse[:], se[:], AF.Ln)

                prod = rop.tile([16, 10], F32)
                tcorr = rop.tile([16, 1], F32)
                nc.vector.tensor_tensor(prod[:], lgt[:], ohb[:],
                                        mybir.AluOpType.mult)
                nc.vector.tensor_reduce(tcorr[:], prod[:], mybir.AxisListType.X,
                                        mybir.AluOpType.add)

                lossv = rop.tile([16, 1], F32)
                accv = rop.tile([16, 1], F32)
                nc.vector.tensor_tensor(lossv[:], lse[:], mx[:],
                                        mybir.AluOpType.add)
                nc.vector.tensor_tensor(lossv[:], lossv[:], tcorr[:],
                                        mybir.AluOpType.subtract)
                nc.vector.tensor_tensor(accv[:], mx[:], tcorr[:],
                                        mybir.AluOpType.is_equal)

                lv2 = rop.tile([128, 2], F32)
                nc.vector.memset(lv2[:], 0.0)
                nc.vector.tensor_copy(lv2[0:16, 0:1], lossv[:])
                nc.vector.tensor_copy(lv2[0:16, 1:2], accv[:])
                ones = rop.tile([128, 1], F32)
                nc.vector.memset(ones[:], 1.0)
                sm_ps = prop.tile([2, 1], F32)
                nc.tensor.matmul(sm_ps[:, :], lv2[:], ones[:],
                                 start=True, stop=True)
                sm = rop.tile([2, 1], F32)
                nc.scalar.activation(sm[:], sm_ps[:, :], AF.Identity, bias=0.0)
                nc.sync.dma_start(out_d[:], sm[:])

    nc.compile()
    return nc


# ---------------- host-side input prep ----------------

def prep_in_maps(inputs):
    DT_np = {"bf16": ml_dtypes.bfloat16, "fp32r": np.float32,
             "fp32": np.float32}[DT_NAME]
    f = np.float32
    x = np.asarray(inputs["x"], f)          # [128, 3, 32, 32]
    y = np.asarray(inputs["y"]).astype(np.int64)  # [128]
    aug_W = np.asarray(inputs["aug_W"], f)  # [32, 3]
    aug_b = np.asarray(inputs["aug_b"], f)  # [32]
    W1 = np.asarray(inputs["W1"], f)        # [4, 128, 33]
    b1 = np.asarray(inputs["b1"], f)        # [4, 128]
    W2 = np.asarray(inputs["W2"], f)        # [4, 128, 128, 3, 3]
    b2 = np.asarray(inputs["b2"], f)        # [4, 128]
    W3 = np.asarray(inputs["W3"], f)        # [4, 32, 128]
    b3 = np.asarray(inputs["b3"], f)        # [4, 32]
    ro_W = np.asarray(inputs["ro_W"], f)    # [10, 32768]
    ro_b = np.asarray(inputs["ro_b"], f)    # [10]

    # xf: [core, 128 = g*32+c (c<3), 4096 = s*1024 + pos]
    xr = x.reshape(NCORES, 4, 4, 3, 1024)          # [core, g, s, c, pos]
    xf = np.zeros((NCORES, 4, 32, 4, 1024), f)     # [core, g, c, s, pos]
    xf[:, :, :3] = xr.transpose(0, 1, 3, 2, 4)
    xf = np.ascontiguousarray(xf.reshape(NCORES, 128, 4096)).astype(DT_np)

    # w1s[p, g, 32g+c, m] = W1[p, m, 1+c]; zero outside group g's strip
    w1T = W1[:, :, 1:].transpose(0, 2, 1)          # [p, c, m]
    w1s = np.zeros((4, 4, 128, 128), f)
    for g in range(4):
        w1s[:, g, 32 * g:32 * g + 32, :] = w1T
    w1s = w1s.astype(DT_np)
    w2s = np.ascontiguousarray(
        W2.transpose(0, 3, 4, 2, 1).reshape(4, 9, 128, 128)).astype(DT_np)
    w3s = np.ascontiguousarray(W3.transpose(0, 2, 1)).astype(DT_np)
    # augw[g, 32g+i, m] = aug_W[m, i] (i<3); zero elsewhere
    augw = np.zeros((4, 128, 32), f)
    for g in range(4):
        augw[g, 32 * g:32 * g + 3, :] = aug_W.T
    augw = augw.astype(DT_np)

    b1e = np.empty((128, 16), f)
    for eidx, (t, piece) in enumerate(_EVAL_TP):
        b1e[:, eidx] = b1[piece] + np.float32(t) * W1[piece][:, 0]
    b2s = np.ascontiguousarray(b2.T)                       # [128, 4]
    b3s = np.ascontiguousarray(-2.0 * np.tile(b3, (1, 4)).T)  # [128, 4] folded
    augb = np.tile(aug_b, 4)[:, None].astype(f)            # [128, 1]

    # row: [128 = t*32+c, q*10+cls] = ro_W[cls, c*1024 + t*256 + q]
    ro4 = ro_W.reshape(10, 32, 4, 256)                     # [cls, c, t, q]
    row = np.ascontiguousarray(
        ro4.transpose(2, 1, 3, 0).reshape(128, 2560))      # [t*32+c, q, cls]
    rob = np.tile(ro_b, (16, 1)).astype(f)                 # [16, 10]

    eye = np.eye(10, dtype=f)
    in_maps = []
    for k in range(NCORES):
        oneh = eye[y[k * BL:(k + 1) * BL]]
        in_maps.append({
            "xf": xf[k], "w1s": w1s, "w2s": w2s, "w3s": w3s, "augw": augw,
            "b1e": b1e, "b2s": b2s, "b3s": b3s, "augb": augb,
            "row": row, "oneh": oneh, "rob": rob,
        })
    return in_maps


_NC_CACHE = {}


def _get_nc(debug=False):
    key = (DT_NAME, debug)
    if key not in _NC_CACHE:
        _NC_CACHE[key] = build_nc(debug)
    return _NC_CACHE[key]


def run(inputs, debug=False, **spmd_kwargs):
    nc = _get_nc(debug)
    in_maps = prep_in_maps(inputs)
    res = run_bass_kernel_spmd(nc, in_maps, core_ids=list(range(NCORES)),
                               **spmd_kwargs)
    loss = sum(r["outv"][0, 0] for r in res.results) / 128.0
    accu = sum(r["outv"][1, 0] for r in res.results)
    out = (np.asarray(loss, np.float32), np.asarray(accu, np.float32))
    return out, res


def kernel(**inputs):
    out, _ = run(inputs)
    return out


# revision 15
# speedup vs baseline: 1.7881x; 1.0184x over previous
"""Trainium2 Bass kernel for nn_NeuralODECNN (RK4 neural-ODE CNN forward).

Self-contained: hardcodes all shapes. Data-parallel over batch across 8
NeuronCores (16 images per core); all params replicated.

Per-core on-chip layouts (B_local=16 images, j = g*4 + s, g=group, s=slot;
pos(j) = s*4 + g = stream index, so the s-major eval order touches
adjacent slots and Ln passes batch over image pairs):
  folded  [128 = g*32 + c, 4096 = s*1024 + y*32 + x]   (z / k / x tensors, c<32)
  wide    [128 = channel,  16384 = pos*1024 + y*32 + x]  (h2, 128 channels)
  h1pad   [128, 16(pos), 34, 34]  zero-padded per image for the 3x3 conv taps

conv1 (32ch -> 128ch 1x1): row-tiled K=32 matmuls (row group = image group)
conv2 (3x3 SAME): 9 accumulating matmuls per output chunk, shifted AP taps
conv3 (128ch -> 32ch 1x1): col-tiled M=32 matmuls into one [128,1024] psum
softplus(x) = Ln(Exp(x)+1) on ScalarE; the Ln pass is batched over image
pairs ([128,2048]) to amortize ACT instruction overhead.
All ACT functions (Exp/Ln/Identity) live in the natural_log_exp_and_others
table set; get_activation_tables is patched (order-preserving membership
strip) so walrus picks that single set instead of ping-ponging between
exp_and_others and natural_log (which cost ~900us in table reloads).
tanh(x)     = 2*recip(1+Exp(-2x)) - 1 (Exp on ScalarE, recip on VectorE)
t-channel of conv1 folded into per-eval biases (b1e) on the host.

The 16 RK4 sub-evaluations run as ONE flat software-pipelined stream:
conv1 leads, conv2 lags LAG images, conv3+combine for a completed slot
are emitted C3LAG images later so their Ln/psum waits hide under conv2
matmuls of later images (and, across the eval boundary, under the next
eval's conv1 matmuls).

readout: z is refolded to zR [128 = t*32+c, 16*256] by per-slot DMAs
issued right after that slot's final combine (overlapping the last eval),
then 256 accumulating fp32 matmuls -> logits psum [10,16]; log-softmax +
onehot loss + argmax-accuracy on device; host sums the 8 per-core
(loss_sum, acc_sum) pairs.
"""

import functools
import os
from contextlib import ExitStack

import ml_dtypes
import numpy as np

import concourse.bacc as bacc
import concourse.hw_specs as hw_specs
import concourse.mybir as mybir
import concourse.tile as tile
from concourse.bass_utils import run_bass_kernel_spmd

F32 = mybir.dt.float32
AF = mybir.ActivationFunctionType

NCORES = 8
BL = 16          # images per core
STEPS = 4        # RK4 steps (= pieces, STEPS_PER_PIECE=1, dt=1)
DT_NAME = os.environ.get("ODE_DT", "bf16")   # bf16 | fp32r | fp32
LAG = 2          # conv2 image lag behind conv1 within the flat stream
C3LAG = 2        # conv3+combine emission lag behind slot completion

# eval schedule: step i evals use t = i + {0,.5,.5,1}, piece = i (k1..k3) or
# min(i+1,3) (k4, since floor(i+1) indexes the next piece)
_EVAL_TP = [(i + dt, i if k < 3 else min(i + 1, 3))
            for i in range(4) for k, dt in enumerate((0.0, 0.5, 0.5, 1.0))]


def _patch_act_tables():
    """Make walrus resolve every ACT function to the one table set that
    contains all of {Exp, Ln, Identity}: strip those functions from the
    other sets so first-match lands on natural_log_exp_and_others.
    Order (and therefore act_func_set_id indices) is preserved; only the
    selection inside bass's insert_act_table_loads changes."""
    orig = hw_specs.get_activation_tables

    @functools.cache
    def gat(arch):
        t = dict(orig(arch))
        pref = "natural_log_exp_and_others"
        if pref not in t:
            return t
        keep = t[pref]
        return {k: (v if k == pref else v - keep) for k, v in t.items()}

    bacc.get_activation_tables = gat


_patch_act_tables()


def _mm_dtype():
    return {"bf16": mybir.dt.bfloat16, "fp32r": F32, "fp32": F32}[DT_NAME]


def build_nc(debug=False):
    DT = _mm_dtype()
    if DT_NAME == "fp32r":
        cast = lambda ap: ap.bitcast(mybir.dt.float32r)  # noqa: E731
    else:
        cast = lambda ap: ap  # noqa: E731

    nc = bacc.Bacc("TRN2")

    xf_d = nc.dram_tensor("xf", [128, 4096], DT, kind="ExternalInput")
    # conv1/aug weights are zero-padded to full K=128 (rows outside the
    # group's 32-partition strip are zero) — row-tiled partial-K matmuls
    # return garbage on this HW path, full-K costs the same N cycles.
    w1_d = nc.dram_tensor("w1s", [4, 4, 128, 128], DT, kind="ExternalInput")
    w2_d = nc.dram_tensor("w2s", [4, 9, 128, 128], DT, kind="ExternalInput")
    w3_d = nc.dram_tensor("w3s", [4, 128, 32], DT, kind="ExternalInput")
    aw_d = nc.dram_tensor("augw", [4, 128, 32], DT, kind="ExternalInput")
    b1_d = nc.dram_tensor("b1e", [128, 16], F32, kind="ExternalInput")
    b2_d = nc.dram_tensor("b2s", [128, 4], F32, kind="ExternalInput")
    b3_d = nc.dram_tensor("b3s", [128, 4], F32, kind="ExternalInput")  # -2*b3 folded
    ab_d = nc.dram_tensor("augb", [128, 1], F32, kind="ExternalInput")
    ro_d = nc.dram_tensor("row", [128, 2560], F32, kind="ExternalInput")
    out_d = nc.dram_tensor("outv", [10, 16], F32, kind="ExternalOutput")
    if debug:
        zf_d = nc.dram_tensor("zf", [128, 4096], F32, kind="ExternalOutput")

    with tile.TileContext(nc) as tc, ExitStack() as ctx:
        sing = ctx.enter_context(tc.tile_pool(name="sing", bufs=1))
        z = sing.tile([128, 4096], F32)
        zin = sing.tile([128, 4096], DT)
        acc = sing.tile([128, 4096], F32)
        w1b = sing.tile([128, 4, 4, 128], DT)
        w2b = sing.tile([128, 4, 9, 128], DT)
        w3b = sing.tile([128, 4, 32], DT)
        awb = sing.tile([128, 4, 32], DT)
        b1b = sing.tile([128, 16], F32)
        b2b = sing.tile([128, 4], F32)
        b3b = sing.tile([128, 4], F32)
        abb = sing.tile([128, 1], F32)
        zR = sing.tile([128, 4096], F32)
        rob_w = sing.tile([128, 2560], F32)
        lgs = sing.tile([10, 16], F32)
        xfb = sing.tile([128, 4096], DT)

        # issue order = need order: aug inputs, conv1, conv2, conv3, readout
        nc.sync.dma_start(xfb[:], xf_d[:])
        nc.sync.dma_start(awb[:], aw_d.rearrange("g i m -> i g m"))
        nc.sync.dma_start(abb[:], ab_d[:])
        nc.sync.dma_start(w1b[:], w1_d.rearrange("p g i m -> i p g m"))
        nc.sync.dma_start(b1b[:], b1_d[:])
        nc.sync.dma_start(w2b[:], w2_d.rearrange("p t i m -> i p t m"))
        nc.sync.dma_start(b2b[:], b2_d[:])
        nc.sync.dma_start(w3b[:], w3_d.rearrange("p i m -> i p m"))
        nc.sync.dma_start(b3b[:], b3_d[:])
        nc.sync.dma_start(rob_w[:], ro_d[:])

        # refold view: zR[t*32+c, (g*4+s)*256 + q] = z[g*32+c, s*1024+t*256+q]
        zv = z.rearrange("p (s t q) -> p s t q", t=4, q=256)
        zRv = zR.rearrange("p (g s q) -> p g s q", s=4, q=256)

        with (
            tc.tile_pool(name="mid", bufs=1) as mid,
            tc.tile_pool(name="p1", bufs=1, space="PSUM") as p1p,
            tc.tile_pool(name="p2", bufs=2, space="PSUM") as p2p,
            tc.tile_pool(name="p3", bufs=1, space="PSUM") as p3p,
            tc.tile_pool(name="stg", bufs=2) as stg,
        ):
            h1pad = mid.tile([128, 16, 34, 34], DT)
            h2b = mid.tile([128, 16384], DT)
            e3b = mid.tile([128, 1024], F32)
            t0b = mid.tile([128, 1024], F32)

            # only the 1-px border needs to be zero; the interior is
            # rewritten by every eval's Ln pass
            nc.vector.memset(h1pad[:, :, 0, :], 0.0)
            nc.vector.memset(h1pad[:, :, 33, :], 0.0)
            nc.vector.memset(h1pad[:, :, 1:33, 0], 0.0)
            nc.vector.memset(h1pad[:, :, 1:33, 33], 0.0)

            # ---- augment: z0 = aug_W @ x + aug_b (col-tiled, zero-pad K) ----
            # uses the double-buffered conv2 psum pool so slot s+1's matmuls
            # overlap slot s's DVE bias-add drain
            for s in range(4):
                ps = p2p.tile([128, 1024], F32, tag="ps2")
                for g in range(4):
                    for h in range(2):
                        n0 = s * 1024 + h * 512
                        nc.tensor.matmul(
                            ps[32 * g:32 * g + 32, h * 512:(h + 1) * 512],
                            cast(awb[:, g, :]),
                            cast(xfb[:, n0:n0 + 512]),
                            start=True, stop=True, tile_position=(0, 32 * g))
                sl = slice(s * 1024, (s + 1) * 1024)
                nc.vector.tensor_scalar(zin[:, sl], ps[:], abb[:, 0:1], None,
                                        mybir.AluOpType.add)
                nc.vector.tensor_scalar(z[:, sl], ps[:], abb[:, 0:1], None,
                                        mybir.AluOpType.add)

            # ---- the 16 RK4 sub-evaluations, one flat pipelined stream ----
            def conv1(i, piece, eidx, st2):
                # image imgs[i] = (i%4)*4 + i//4; h1pad slot = i
                g, s = i % 4, i // 4
                ps1 = p1p.tile([128, 1024], F32, tag="ps1")
                for h in range(2):
                    n0 = s * 1024 + h * 512
                    nc.tensor.matmul(
                        ps1[:, h * 512:(h + 1) * 512],
                        cast(w1b[:, piece, g, :]),
                        cast(zin[:, n0:n0 + 512]),
                        start=True, stop=True)
                half = i % 2
                nc.scalar.activation(st2[:, half * 1024:(half + 1) * 1024],
                                     ps1[:], AF.Exp, bias=b1b[:, eidx:eidx + 1])
                if half == 1:
                    nc.scalar.activation(
                        h1pad[:, i - 1:i + 1, 1:33, 1:33],
                        st2.rearrange("p (j a b) -> p j a b", a=32, b=32),
                        AF.Ln, bias=1.0)

            def conv2(i, piece, st2c):
                ps2 = p2p.tile([128, 1024], F32, tag="ps2")
                for tap in range(9):
                    dy, dx = tap // 3, tap % 3
                    for h in range(2):
                        y0 = h * 16 + dy
                        nc.tensor.matmul(
                            ps2[:, h * 512:(h + 1) * 512],
                            cast(w2b[:, piece, tap, :]),
                            cast(h1pad[:, i, y0:y0 + 16, dx:dx + 32]),
                            start=(tap == 0), stop=(tap == 8))
                half = i % 2
                nc.scalar.activation(st2c[:, half * 1024:(half + 1) * 1024],
                                     ps2[:], AF.Exp, bias=b2b[:, piece:piece + 1])
                if half == 1:
                    nc.scalar.activation(h2b[:, (i - 1) * 1024:(i + 1) * 1024],
                                         st2c[:], AF.Ln, bias=1.0)

            def conv3(s, piece):
                # both 512-chunks of the slot into one [128,1024] psum tile
                ps3 = p3p.tile([128, 1024], F32, tag="ps3")
                for half in range(2):
                    for g in range(4):
                        n0 = (s * 4 + g) * 1024 + half * 512
                        nc.tensor.matmul(
                            ps3[32 * g:32 * g + 32,
                                half * 512:(half + 1) * 512],
                            cast(w3b[:, piece, :]), cast(h2b[:, n0:n0 + 512]),
                            start=True, stop=True, tile_position=(0, 32 * g))
                nc.scalar.activation(e3b[:], ps3[:], AF.Exp, scale=-2.0,
                                     bias=b3b[:, piece:piece + 1])

            def dve_combine(e, s, last_step):
                # k = tanh = 2*r - 1 with r = 1/(1+exp(-2x)); dt = 1
                sl = slice(s * 1024, (s + 1) * 1024)
                ts, tt = nc.vector.tensor_scalar, nc.vector.tensor_tensor
                add, sub, mult = (mybir.AluOpType.add, mybir.AluOpType.subtract,
                                  mybir.AluOpType.mult)
                ts(e3b[:], e3b[:], 1.0, None, add)               # u = e + 1
                # u in [1, ~5e8]: no 0/denorm/inf, so the ~51-ULP fast
                # reciprocal (5x faster than reciprocal()) is safe here
                nc.vector.reciprocal_approx_fast(e3b[:], e3b[:])  # r = 1/u
                r, t0 = e3b[:], t0b[:]
                if e == 0:
                    ts(acc[:, sl], r, 2.0, -1.0, mult, add)      # acc = k1
                    ts(t0, r, 0.5, None, sub)                    # k1/2 = r - 1/2
                    tt(zin[:, sl], z[:, sl], t0, add)            # zmid = z + k1/2
                elif e == 1:
                    ts(t0, r, 0.5, None, sub)                    # k2/2
                    tt(zin[:, sl], z[:, sl], t0, add)            # zmid = z + k2/2
                    ts(t0, t0, 4.0, None, mult)                  # 2*k2
                    tt(acc[:, sl], acc[:, sl], t0, add)
                elif e == 2:
                    ts(t0, r, 2.0, -1.0, mult, add)              # k3
                    tt(zin[:, sl], z[:, sl], t0, add)            # zmid = z + k3
                    ts(t0, t0, 2.0, None, mult)                  # 2*k3
                    tt(acc[:, sl], acc[:, sl], t0, add)
                else:
                    ts(t0, r, 2.0, -1.0, mult, add)              # k4
                    tt(acc[:, sl], acc[:, sl], t0, add)
                    ts(t0, acc[:, sl], 1.0 / 6.0, None, mult)
                    tt(z[:, sl], z[:, sl], t0, add)              # z += acc/6
                    if not last_step:
                        nc.vector.tensor_copy(zin[:, sl], z[:, sl])

            def refold_slot(s):
                for g in range(4):
                    for t in range(4):
                        nc.sync.dma_start(zRv[32 * t:32 * t + 32, g, s, :],
                                          zv[32 * g:32 * g + 32, s, t, :])

            n_evals = int(os.environ.get("ODE_NEVALS", "16"))
            total = n_evals * BL
            pending = {}
            c2done = [[0] * 4 for _ in range(n_evals)]
            refolded = [False] * 4

            def fire_c3(ev, s):
                step, e = ev // 4, ev % 4
                piece = _EVAL_TP[ev][1]
                conv3(s, piece)
                dve_combine(e, s, last_step=(step == STEPS - 1))
                if ev == n_evals - 1 and n_evals == 16:
                    refold_slot(s)
                    refolded[s] = True

            st1 = st2c = None
            for I in range(total + LAG):
                for ev, s in pending.pop(I, ()):
                    fire_c3(ev, s)
                if I < total:
                    ev1, i1 = divmod(I, BL)
                    if i1 % 2 == 0:
                        st1 = stg.tile([128, 2048], F32, tag="st")
                    conv1(i1, _EVAL_TP[ev1][1], ev1, st1)
                J = I - LAG
                if J >= 0:
                    ev2, i2 = divmod(J, BL)
                    if i2 % 2 == 0:
                        st2c = stg.tile([128, 2048], F32, tag="st2")
                    conv2(i2, _EVAL_TP[ev2][1], st2c)
                    s = i2 // 4
                    c2done[ev2][s] += 1
                    if c2done[ev2][s] == 4:
                        pending.setdefault(I + C3LAG, []).append((ev2, s))
            for I in range(total + LAG, total + LAG + C3LAG + 1):
                for ev, s in pending.pop(I, ()):
                    fire_c3(ev, s)
            assert not pending

            if not all(refolded):   # debug fallback (ODE_NEVALS < 16)
                for s in range(4):
                    if not refolded[s]:
                        refold_slot(s)

            # ---- readout: logits = ro_W @ z, two accumulation chains ----
            # Chain A (slots 0-2, 12 images) depends only on already-refolded
            # zR, so its 256 matmuls hide the last slot's combine + refold
            # latency; chain B (slot 3, 4 images) runs after refold(3).
            # Column order is scrambled (A: g*3+s, B: 12+g); host unscrambles.
            lgp = p1p.tile([128, 1024], F32, tag="ps1")
            zRgs = zR.rearrange("p (g s q) -> p g s q", s=4, q=256)
            for q in range(256):
                nc.tensor.matmul(lgp[0:10, 0:12], rob_w[:, 10 * q:10 * q + 10],
                                 zRgs[:, :, 0:3, q],
                                 start=(q == 0), stop=(q == 255))
            for q in range(256):
                nc.tensor.matmul(lgp[0:10, 12:16], rob_w[:, 10 * q:10 * q + 10],
                                 zRgs[:, :, 3, q],
                                 start=(q == 0), stop=(q == 255))
            nc.scalar.activation(lgs[:], lgp[0:10, 0:16], AF.Identity, bias=0.0)
            nc.sync.dma_start(out_d[:], lgs[:])
            if debug:
                nc.sync.dma_start(zf_d[:], z[:])

    nc.compile()
    return nc


# ---------------- host-side input prep ----------------

def prep_in_maps(inputs):
    DT_np = {"bf16": ml_dtypes.bfloat16, "fp32r": np.float32,
             "fp32": np.float32}[DT_NAME]
    f = np.float32
    x = np.asarray(inputs["x"], f)          # [128, 3, 32, 32]
    y = np.asarray(inputs["y"]).astype(np.int64)  # [128]
    aug_W = np.asarray(inputs["aug_W"], f)  # [32, 3]
    aug_b = np.asarray(inputs["aug_b"], f)  # [32]
    W1 = np.asarray(inputs["W1"], f)        # [4, 128, 33]
    b1 = np.asarray(inputs["b1"], f)        # [4, 128]
    W2 = np.asarray(inputs["W2"], f)        # [4, 128, 128, 3, 3]
    b2 = np.asarray(inputs["b2"], f)        # [4, 128]
    W3 = np.asarray(inputs["W3"], f)        # [4, 32, 128]
    b3 = np.asarray(inputs["b3"], f)        # [4, 32]
    ro_W = np.asarray(inputs["ro_W"], f)    # [10, 32768]
    ro_b = np.asarray(inputs["ro_b"], f)    # [10]

    # xf: [core, 128 = g*32+c (c<3), 4096 = s*1024 + pos]
    xr = x.reshape(NCORES, 4, 4, 3, 1024)          # [core, g, s, c, pos]
    xf = np.zeros((NCORES, 4, 32, 4, 1024), f)     # [core, g, c, s, pos]
    xf[:, :, :3] = xr.transpose(0, 1, 3, 2, 4)
    xf = np.ascontiguousarray(xf.reshape(NCORES, 128, 4096)).astype(DT_np)

    # w1s[p, g, 32g+c, m] = W1[p, m, 1+c]; zero outside group g's strip
    w1T = W1[:, :, 1:].transpose(0, 2, 1)          # [p, c, m]
    w1s = np.zeros((4, 4, 128, 128), f)
    for g in range(4):
        w1s[:, g, 32 * g:32 * g + 32, :] = w1T
    w1s = w1s.astype(DT_np)
    w2s = np.ascontiguousarray(
        W2.transpose(0, 3, 4, 2, 1).reshape(4, 9, 128, 128)).astype(DT_np)
    w3s = np.ascontiguousarray(W3.transpose(0, 2, 1)).astype(DT_np)
    # augw[g, 32g+i, m] = aug_W[m, i] (i<3); zero elsewhere
    augw = np.zeros((4, 128, 32), f)
    for g in range(4):
        augw[g, 32 * g:32 * g + 3, :] = aug_W.T
    augw = augw.astype(DT_np)

    b1e = np.empty((128, 16), f)
    for eidx, (t, piece) in enumerate(_EVAL_TP):
        b1e[:, eidx] = b1[piece] + np.float32(t) * W1[piece][:, 0]
    b2s = np.ascontiguousarray(b2.T)                       # [128, 4]
    b3s = np.ascontiguousarray(-2.0 * np.tile(b3, (1, 4)).T)  # [128, 4] folded
    augb = np.tile(aug_b, 4)[:, None].astype(f)            # [128, 1]

    # row: [128 = t*32+c, q*10+cls] = ro_W[cls, c*1024 + t*256 + q]
    ro4 = ro_W.reshape(10, 32, 4, 256)                     # [cls, c, t, q]
    row = np.ascontiguousarray(
        ro4.transpose(2, 1, 3, 0).reshape(128, 2560))      # [t*32+c, q, cls]

    in_maps = []
    for k in range(NCORES):
        in_maps.append({
            "xf": xf[k], "w1s": w1s, "w2s": w2s, "w3s": w3s, "augw": augw,
            "b1e": b1e, "b2s": b2s, "b3s": b3s, "augb": augb,
            "row": row,
        })
    return in_maps


_NC_CACHE = {}


def _get_nc(debug=False):
    key = (DT_NAME, debug)
    if key not in _NC_CACHE:
        _NC_CACHE[key] = build_nc(debug)
    return _NC_CACHE[key]


def run(inputs, debug=False, **spmd_kwargs):
    nc = _get_nc(debug)
    in_maps = prep_in_maps(inputs)
    res = run_bass_kernel_spmd(nc, in_maps, core_ids=list(range(NCORES)),
                               **spmd_kwargs)
    # host epilogue (unshard glue): unscramble per-core logits, then
    # log-softmax loss + argmax accuracy in fp32, summed over the 8 cores
    f = np.float32
    y = np.asarray(inputs["y"]).astype(np.int64)
    ro_b = np.asarray(inputs["ro_b"], f)
    loss_sum, acc_sum = 0.0, 0.0
    for k in range(NCORES):
        arr = np.asarray(res.results[k]["outv"], f)     # [10, 16] scrambled
        lg = np.empty((BL, 10), f)
        for g in range(4):
            lg[g * 4 + 0:g * 4 + 3] = arr[:, g * 3:g * 3 + 3].T
            lg[g * 4 + 3] = arr[:, 12 + g]
        lg = lg + ro_b
        yk = y[k * BL:(k + 1) * BL]
        m = lg.max(axis=1)
        lse = np.log(np.exp(lg - m[:, None]).sum(axis=1)).astype(f) + m
        loss_sum += float((lse - lg[np.arange(BL), yk]).sum())
        acc_sum += float((lg.argmax(axis=1) == yk).sum())
    out = (np.float32(loss_sum / 128.0), np.float32(acc_sum))
    return out, res


def kernel(**inputs):
    out, _ = run(inputs)
    return out
